# revision 17
# baseline (speedup 1.0000x reference)
"""Trainium2 Bass kernel for nn_BlockDiagonalLinear_text (hyperbolic block-diag linear).

Math: every per-row op in the reference is a scalar row-scaling, so
  out = alpha_row * y,   y = x @ blockdiag(W_1..W_16).T
and the whole tanh/artanh chain collapses exactly (monotonicity:
artanh(clip(tanh(t), max=z)) == clip(t, max=artanh(z))) to
  alpha = min(1, A/||x||, B/||y||) * [||y||>0]
  A = 10*artanh(1-1e-5), B = 10*artanh(0.999)
(validated numerically against the reference to ~1e-5 in f64).

Sharding: data-parallel over rows; 8192 rows -> 8 cores x 1024 rows;
weights replicated.

Per-core kernel, per 128-row tile (all matmul data bf16):
  host supplies x^T pre-transposed/cast per tile ([tile, k, kchunk, row],
  8KB contiguous per partition -> full-rate DMA). PE per tile: 32-chunk
  Gram accumulation (diag = ||x_row||^2) then 32 block matmuls (the PE
  cannot keep two multi-instruction accumulation groups open at once).
  PSUM y copies cast to bf16 (1 ACT + 3 DVE); ||y||^2 = one big ACT
  Square-with-accum over SBUF y (per-PSUM-chunk on the last tile to
  shorten the tail); 5-op DVE min() chain -> alpha; in-place bf16 scale;
  outputs DMA'd on the GPSIMD SWDGE ring (inputs on SP, weights on ACT).
  sqrt/square/copy live in one ACT table set: no table thrash.
"""
import sys
import numpy as np
import ml_dtypes

for _p in ("/opt/trn_rl_repo", "/root/.axon_site/_ro/trn_rl_repo"):
    if _p not in sys.path:
        sys.path.append(_p)

import concourse.bass as bass
import concourse.bacc as bacc
import concourse.mybir as mybir
from concourse import tile
from concourse.bass_utils import run_bass_kernel_spmd

R, BS = 16, 256           # 16 diagonal blocks of 256x256
D = R * BS                # 4096
P = 128                   # partitions
NCH = D // P              # 32 contraction chunks of 128
N_CORES = 8
ROWS_TOTAL = 4 * 2048     # 8192
ROWS_CORE = ROWS_TOTAL // N_CORES   # 1024
NT = ROWS_CORE // P       # 8 tiles of 128 rows per core
WCOLS = D * 2             # 8192 weight cols: chunk kc -> [k_local, j(256)]

f32 = mybir.dt.float32
bf16 = mybir.dt.bfloat16
AF = mybir.ActivationFunctionType
OP = mybir.AluOpType
bfnp = ml_dtypes.bfloat16

# alpha = min(1, A61/||x||, B38/||y||): exact collapse of the reference's
# expmap/mobius/project/logmap chain (f32 clip constants).
_CLIP1 = float(np.float32(1.0) - np.float32(1e-5))            # 0.99999
_MAXN = float(np.float32(1.0 - 1e-3) / np.float32(0.1))       # 9.99
_CLIP2 = float(np.float32(0.1) * np.float32(_MAXN))           # 0.999
A61 = float(10.0 * np.arctanh(np.float64(_CLIP1)))            # 61.0303...
B38 = float(10.0 * np.arctanh(np.float64(_CLIP2)))            # 38.0020...


def build_nc():
    nc = bacc.Bacc()
    xt_d = nc.declare_dram_parameter("xt", [NT, P, NCH, P], bf16,
                                     isOutput=False)
    w_d = nc.declare_dram_parameter("w", [P, WCOLS], bf16, isOutput=False)
    idm_d = nc.declare_dram_parameter("idm", [P, P], f32, isOutput=False)
    out_d = nc.declare_dram_parameter("out", [ROWS_CORE, D], bf16,
                                      isOutput=True)

    with tile.TileContext(nc) as tc:
        with (
            tc.tile_pool(name="wpool", bufs=1) as wpool,
            tc.tile_pool(name="xpool", bufs=2) as xpool,
            tc.tile_pool(name="ypool", bufs=3) as ypool,
            tc.tile_pool(name="opool", bufs=2) as opool,
            tc.tile_pool(name="scr", bufs=2) as scr,
            tc.tile_pool(name="stats", bufs=3) as stats,
            tc.tile_pool(name="psg", bufs=2, space="PSUM") as psg,
            tc.tile_pool(name="psy", bufs=3, space="PSUM") as psy,
        ):
            V = nc.vector
            w_sb = wpool.tile([P, WCOLS], bf16, name="w_sb")
            idm = wpool.tile([P, P], f32, name="idm")
            # weights on the ACT HWDGE ring; xt/idm/out on the SP ring.
            # (concurrent DMAs round-robin per packet across the SDMA
            # engines, so the first tile's load is split into 4 slices -
            # the gram phase starts after the first 256KB - and prefetch
            # depth stays at 2 so tile0 isn't delayed by deep prefetch.)
            for s in range(4):
                qw = WCOLS // 4
                nc.scalar.dma_start(out=w_sb[:, s * qw:(s + 1) * qw],
                                    in_=w_d[:, s * qw:(s + 1) * qw])

            # one-tile-lag software pipeline: tile i-1's qy/chain/scale/DMA
            # are emitted AFTER tile i's copies so the next tile's g0 copy
            # never queues behind the previous tile's 3.6us Square on ACT.
            prev = None

            def finish(s, last):
                i, y_sb, st2, pys = s
                sq = scr.tile([P, D], bf16, tag="sq", name=f"sq_{i}")
                if not last:
                    # one big Square+accum over SBUF y (fewest ACT ops)
                    nc.scalar.activation(sq[:], y_sb[:], AF.Square,
                                         accum_out=st2[:, 1:2])
                else:
                    # short tail: qy split across ACT and DVE in parallel
                    qyp = stats.tile([P, 3], f32, tag="qyp", name=f"qyp_{i}")
                    for g in range(2):
                        nc.scalar.activation(
                            sq[:, g * 1024:(g + 1) * 1024],
                            y_sb[:, g * 1024:(g + 1) * 1024],
                            AF.Square, accum_out=qyp[:, g:g + 1])
                    for g in range(2, 4):
                        V.tensor_tensor(
                            out=sq[:, g * 1024:(g + 1) * 1024],
                            in0=y_sb[:, g * 1024:(g + 1) * 1024],
                            in1=y_sb[:, g * 1024:(g + 1) * 1024], op=OP.mult)
                    V.tensor_reduce(qyp[:, 2:3], sq[:, 2048:4096],
                                    axis=mybir.AxisListType.X, op=OP.add)
                    V.tensor_reduce(st2[:, 1:2], qyp[:],
                                    axis=mybir.AxisListType.X, op=OP.add)

                # alpha = min(1, A61/u, B38/yn) * [qy>0]
                mask = stats.tile([P, 1], f32, tag="mask", name=f"mk_{i}")
                V.tensor_scalar(out=mask[:], in0=st2[:, 1:2], scalar1=0.0,
                                scalar2=None, op0=OP.is_gt)
                s2 = stats.tile([P, 2], f32, tag="s2", name=f"s2_{i}")
                nc.scalar.activation(s2[:], st2[:], AF.Sqrt)
                rc = stats.tile([P, 2], f32, tag="rc", name=f"rc_{i}")
                V.reciprocal(rc[:], s2[:])
                ta = stats.tile([P, 1], f32, tag="ta", name=f"ta_{i}")
                V.tensor_scalar_mul(ta[:], rc[:, 0:1], A61)
                al0 = stats.tile([P, 1], f32, tag="al0", name=f"al0_{i}")
                V.scalar_tensor_tensor(out=al0[:], in0=rc[:, 1:2],
                                       scalar=B38, in1=ta[:],
                                       op0=OP.mult, op1=OP.min)
                alm = stats.tile([P, 1], f32, tag="alm", name=f"alm_{i}")
                V.scalar_tensor_tensor(out=alm[:], in0=al0[:], scalar=1.0,
                                       in1=mask[:], op0=OP.min, op1=OP.mult)
                # bf16 scale on the (otherwise idle) Pool engine via a
                # stride-0 broadcast of alm; SP-ring out-DMA
                o_sb = opool.tile([P, D], bf16, tag="o", name=f"o_{i}")
                nc.gpsimd.tensor_tensor(out=o_sb[:], in0=y_sb[:],
                                        in1=alm[:, 0:1].broadcast_to((P, D)),
                                        op=OP.mult)
                nc.sync.dma_start(out=out_d[i * P:(i + 1) * P, :],
                                  in_=o_sb[:])

            for i in range(NT):
                xt = xpool.tile([P, NCH, P], bf16, tag="xt", name=f"xt_{i}")
                if i == 0:
                    for s in range(4):
                        nc.sync.dma_start(
                            out=xt[:, s * 8:(s + 1) * 8, :],
                            in_=xt_d[i, :, s * 8:(s + 1) * 8, :])
                    nc.sync.dma_start(out=idm[:], in_=idm_d[:])
                else:
                    nc.sync.dma_start(out=xt[:], in_=xt_d[i])

                # ---- PE: gram phase, then block-matmul phase ----
                gram = psg.tile([P, P], f32, tag="gram", name=f"g_{i}")
                for kc in range(NCH):
                    nc.tensor.matmul(gram[:], xt[:, kc, :], xt[:, kc, :],
                                     start=(kc == 0), stop=(kc == NCH - 1))
                pys = [psy.tile([P, 1024], f32, tag="py", name=f"py_{i}_{g}")
                       for g in range(4)]
                for kc in range(NCH):
                    r = kc // 2
                    g, q = r // 4, r % 4
                    nc.tensor.matmul(
                        pys[g][:, q * BS:(q + 1) * BS],
                        xt[:, kc, :], w_sb[:, kc * BS:(kc + 1) * BS],
                        start=(kc % 2 == 0), stop=(kc % 2 == 1),
                    )

                # ---- PSUM -> SBUF copies (cast bf16) + qx diag ----
                # DVE takes g0 FIRST so the within-tile psy-slot reuse
                # (g3 reuses g0's slot, ring of 3) never stalls the PE;
                # ACT takes g1; diag slots between DVE copies.
                st2 = stats.tile([P, 2], f32, tag="st2", name=f"st2_{i}")
                dsc = scr.tile([P, P], f32, tag="dsc", name=f"dsc_{i}")
                y_sb = ypool.tile([P, D], bf16, tag="y", name=f"y_{i}")
                V.tensor_copy(y_sb[:, 0:1024], pys[0][:])
                nc.scalar.copy(y_sb[:, 1024:2048], pys[1][:])
                V.tensor_tensor(out=dsc[:], in0=gram[:], in1=idm[:],
                                op=OP.mult)
                V.tensor_reduce(st2[:, 0:1], dsc[:],
                                axis=mybir.AxisListType.X, op=OP.add)
                V.tensor_copy(y_sb[:, 2048:3072], pys[2][:])
                V.tensor_copy(y_sb[:, 3072:4096], pys[3][:])

                if prev is not None:
                    finish(prev, last=False)
                prev = (i, y_sb, st2, pys)

            finish(prev, last=True)
    nc.finalize()
    return nc


_NC = None


def _get_nc():
    global _NC
    if _NC is None:
        _NC = build_nc()
    return _NC


def _prep_weights(weights: np.ndarray) -> np.ndarray:
    # w_sb[p, kc*256 + j] = W[r, j, k], k = kc*128 + p, kc = 2r + c
    wt = (weights.astype(np.float32).transpose(0, 2, 1)      # [r, k, j]
          .reshape(R, 2, P, BS).transpose(2, 0, 1, 3)        # [p, r, c, j]
          .reshape(P, WCOLS))
    return np.ascontiguousarray(wt.astype(bfnp))


def _in_maps(x, weights):
    xf = np.ascontiguousarray(x, dtype=np.float32).reshape(ROWS_TOTAL, D)
    xb = xf.astype(bfnp)
    wid = _prep_weights(np.asarray(weights))
    idm = np.eye(P, dtype=np.float32)
    maps = []
    for c in range(N_CORES):
        xc = xb[c * ROWS_CORE:(c + 1) * ROWS_CORE]           # [1024, 4096]
        # xt[t, p, kc, row] = xc[t*128 + row, kc*128 + p]
        xt = np.ascontiguousarray(
            xc.reshape(NT, P, NCH, P).transpose(0, 3, 2, 1))
        maps.append({"xt": xt, "w": wid, "idm": idm})
    return maps


def kernel(x: np.ndarray, weights: np.ndarray) -> np.ndarray:
    nc = _get_nc()
    res = run_bass_kernel_spmd(nc, _in_maps(x, weights), list(range(N_CORES)))
    out = np.concatenate(
        [np.asarray(res.results[i]["out"]) for i in range(N_CORES)], axis=0)
    return out.reshape(x.shape).astype(np.float32)


def run_traced(x, weights, trace_dir):
    """test.py only: run with NTFF tracing, artifacts into trace_dir."""
    return run_bass_kernel_spmd(
        _get_nc(), _in_maps(x, weights), list(range(N_CORES)),
        trace=True, tmpdir=trace_dir)


if __name__ == "__main__":
    xs = np.random.randn(4, 2048, D).astype(np.float32)
    ws = (np.broadcast_to(np.eye(BS, dtype=np.float32), (16, BS, BS))
          + 0.02 * np.random.randn(16, BS, BS).astype(np.float32))
    o = kernel(xs, ws)
    print("kernel ran, out shape", o.shape, o.dtype)


# revision 18
# speedup vs baseline: 1.5060x; 1.5060x over previous
"""Trainium2 Bass kernel for nn_BlockDiagonalLinear_text (hyperbolic block-diag linear).

Math: every per-row op in the reference is a scalar row-scaling, so
  out = alpha_row * y,   y = x @ blockdiag(W_1..W_16).T
and the whole tanh/artanh chain collapses exactly (monotonicity:
artanh(clip(tanh(t), max=z)) == clip(t, max=artanh(z))) to
  alpha = min(1, A/||x||, B/||y||) * [||y||>0]
  A = 10*artanh(1-1e-5), B = 10*artanh(0.999)
(validated numerically against the reference to ~1e-5 in f64).

Sharding: data-parallel over rows; 8192 rows -> 8 cores x 1024 rows;
weights replicated.

Per-core kernel, per 128-row tile (all matmul data bf16):
  host supplies x^T pre-transposed/cast per tile ([tile, k, kchunk, row],
  8KB contiguous per partition -> full-rate DMA). PE per tile: 32-chunk
  Gram accumulation (diag = ||x_row||^2) then 32 block matmuls (the PE
  cannot keep two multi-instruction accumulation groups open at once).
  PSUM y copies cast to bf16 (1 ACT + 3 DVE); ||y||^2 = one big ACT
  Square-with-accum over SBUF y (per-PSUM-chunk on the last tile to
  shorten the tail); 5-op DVE min() chain -> alpha; in-place bf16 scale;
  outputs DMA'd on the GPSIMD SWDGE ring (inputs on SP, weights on ACT).
  sqrt/square/copy live in one ACT table set: no table thrash.
"""
import sys
import numpy as np
import ml_dtypes

for _p in ("/opt/trn_rl_repo", "/root/.axon_site/_ro/trn_rl_repo"):
    if _p not in sys.path:
        sys.path.append(_p)

import concourse.bass as bass
import concourse.bacc as bacc
import concourse.mybir as mybir
from concourse import tile
from concourse.bass_utils import run_bass_kernel_spmd

R, BS = 16, 256           # 16 diagonal blocks of 256x256
D = R * BS                # 4096
P = 128                   # partitions
NCH = D // P              # 32 contraction chunks of 128
N_CORES = 8
ROWS_TOTAL = 4 * 2048     # 8192
ROWS_CORE = ROWS_TOTAL // N_CORES   # 1024
NT = ROWS_CORE // P       # 8 tiles of 128 rows per core
WCOLS = D * 2             # 8192 weight cols: chunk kc -> [k_local, j(256)]

f32 = mybir.dt.float32
bf16 = mybir.dt.bfloat16
AF = mybir.ActivationFunctionType
OP = mybir.AluOpType
bfnp = ml_dtypes.bfloat16

# alpha = min(1, A61/||x||, B38/||y||): exact collapse of the reference's
# expmap/mobius/project/logmap chain (f32 clip constants).
_CLIP1 = float(np.float32(1.0) - np.float32(1e-5))            # 0.99999
_MAXN = float(np.float32(1.0 - 1e-3) / np.float32(0.1))       # 9.99
_CLIP2 = float(np.float32(0.1) * np.float32(_MAXN))           # 0.999
A61 = float(10.0 * np.arctanh(np.float64(_CLIP1)))            # 61.0303...
B38 = float(10.0 * np.arctanh(np.float64(_CLIP2)))            # 38.0020...


def build_nc():
    nc = bacc.Bacc()
    xt_d = nc.declare_dram_parameter("xt", [NT, P, NCH, P], bf16,
                                     isOutput=False)
    w_d = nc.declare_dram_parameter("w", [P, WCOLS], bf16, isOutput=False)
    idm_d = nc.declare_dram_parameter("idm", [P, P], f32, isOutput=False)
    out_d = nc.declare_dram_parameter("out", [ROWS_CORE, D], bf16,
                                      isOutput=True)

    with tile.TileContext(nc) as tc:
        with (
            tc.tile_pool(name="wpool", bufs=1) as wpool,
            tc.tile_pool(name="xpool", bufs=2) as xpool,
            tc.tile_pool(name="ypool", bufs=3) as ypool,
            tc.tile_pool(name="opool", bufs=2) as opool,
            tc.tile_pool(name="scr", bufs=2) as scr,
            tc.tile_pool(name="stats", bufs=3) as stats,
            tc.tile_pool(name="psg", bufs=2, space="PSUM") as psg,
            tc.tile_pool(name="psy", bufs=3, space="PSUM") as psy,
        ):
            V = nc.vector
            w_sb = wpool.tile([P, WCOLS], bf16, name="w_sb")
            idm = wpool.tile([P, P], f32, name="idm")
            # weights on the ACT HWDGE ring; xt/idm/out on the SP ring.
            # (concurrent DMAs round-robin per packet across the SDMA
            # engines, so the first tile's load is split into 4 slices -
            # the gram phase starts after the first 256KB - and prefetch
            # depth stays at 2 so tile0 isn't delayed by deep prefetch.)
            for s in range(4):
                qw = WCOLS // 4
                nc.scalar.dma_start(out=w_sb[:, s * qw:(s + 1) * qw],
                                    in_=w_d[:, s * qw:(s + 1) * qw])

            # one-tile-lag software pipeline: tile i-1's qy/chain/scale/DMA
            # are emitted AFTER tile i's copies so the next tile's g0 copy
            # never queues behind the previous tile's 3.6us Square on ACT.
            prev = None

            def finish(s, last):
                i, y_sb, st2, pys = s
                sq = scr.tile([P, D], bf16, tag="sq", name=f"sq_{i}")
                if not last:
                    # one big Square+accum over SBUF y (fewest ACT ops)
                    nc.scalar.activation(sq[:], y_sb[:], AF.Square,
                                         accum_out=st2[:, 1:2])
                else:
                    # short tail: qy split across ACT and DVE in parallel
                    qyp = stats.tile([P, 3], f32, tag="qyp", name=f"qyp_{i}")
                    for g in range(2):
                        nc.scalar.activation(
                            sq[:, g * 1024:(g + 1) * 1024],
                            y_sb[:, g * 1024:(g + 1) * 1024],
                            AF.Square, accum_out=qyp[:, g:g + 1])
                    for g in range(2, 4):
                        V.tensor_tensor(
                            out=sq[:, g * 1024:(g + 1) * 1024],
                            in0=y_sb[:, g * 1024:(g + 1) * 1024],
                            in1=y_sb[:, g * 1024:(g + 1) * 1024], op=OP.mult)
                    V.tensor_reduce(qyp[:, 2:3], sq[:, 2048:4096],
                                    axis=mybir.AxisListType.X, op=OP.add)
                    V.tensor_reduce(st2[:, 1:2], qyp[:],
                                    axis=mybir.AxisListType.X, op=OP.add)

                # alpha = min(1, A61/u, B38/yn) * [qy>0]
                mask = stats.tile([P, 1], f32, tag="mask", name=f"mk_{i}")
                V.tensor_scalar(out=mask[:], in0=st2[:, 1:2], scalar1=0.0,
                                scalar2=None, op0=OP.is_gt)
                s2 = stats.tile([P, 2], f32, tag="s2", name=f"s2_{i}")
                nc.scalar.activation(s2[:], st2[:], AF.Sqrt)
                rc = stats.tile([P, 2], f32, tag="rc", name=f"rc_{i}")
                V.reciprocal(rc[:], s2[:])
                ta = stats.tile([P, 1], f32, tag="ta", name=f"ta_{i}")
                V.tensor_scalar_mul(ta[:], rc[:, 0:1], A61)
                al0 = stats.tile([P, 1], f32, tag="al0", name=f"al0_{i}")
                V.scalar_tensor_tensor(out=al0[:], in0=rc[:, 1:2],
                                       scalar=B38, in1=ta[:],
                                       op0=OP.mult, op1=OP.min)
                alm = stats.tile([P, 1], f32, tag="alm", name=f"alm_{i}")
                V.scalar_tensor_tensor(out=alm[:], in0=al0[:], scalar=1.0,
                                       in1=mask[:], op0=OP.min, op1=OP.mult)
                # bf16 scale (4x DVE mode; Pool measures 8.4us for this op
                # and its SBUF-port contention slows every other engine);
                # SP-ring out-DMA
                o_sb = opool.tile([P, D], bf16, tag="o", name=f"o_{i}")
                V.tensor_scalar(out=o_sb[:], in0=y_sb[:], scalar1=alm[:],
                                scalar2=None, op0=OP.mult)
                nc.sync.dma_start(out=out_d[i * P:(i + 1) * P, :],
                                  in_=o_sb[:])

            for i in range(NT):
                xt = xpool.tile([P, NCH, P], bf16, tag="xt", name=f"xt_{i}")
                if i == 0:
                    for s in range(4):
                        nc.sync.dma_start(
                            out=xt[:, s * 8:(s + 1) * 8, :],
                            in_=xt_d[i, :, s * 8:(s + 1) * 8, :])
                    nc.sync.dma_start(out=idm[:], in_=idm_d[:])
                else:
                    nc.sync.dma_start(out=xt[:], in_=xt_d[i])

                # ---- PE: gram phase, then block-matmul phase ----
                gram = psg.tile([P, P], f32, tag="gram", name=f"g_{i}")
                for kc in range(NCH):
                    nc.tensor.matmul(gram[:], xt[:, kc, :], xt[:, kc, :],
                                     start=(kc == 0), stop=(kc == NCH - 1))
                pys = [psy.tile([P, 1024], f32, tag="py", name=f"py_{i}_{g}")
                       for g in range(4)]
                for kc in range(NCH):
                    r = kc // 2
                    g, q = r // 4, r % 4
                    nc.tensor.matmul(
                        pys[g][:, q * BS:(q + 1) * BS],
                        xt[:, kc, :], w_sb[:, kc * BS:(kc + 1) * BS],
                        start=(kc % 2 == 0), stop=(kc % 2 == 1),
                    )

                # ---- PSUM -> SBUF copies (cast bf16) + qx diag ----
                # DVE takes g0 FIRST so the within-tile psy-slot reuse
                # (g3 reuses g0's slot, ring of 3) never stalls the PE;
                # ACT takes g1; diag slots between DVE copies.
                st2 = stats.tile([P, 2], f32, tag="st2", name=f"st2_{i}")
                dsc = scr.tile([P, P], f32, tag="dsc", name=f"dsc_{i}")
                y_sb = ypool.tile([P, D], bf16, tag="y", name=f"y_{i}")
                V.tensor_copy(y_sb[:, 0:1024], pys[0][:])
                nc.scalar.copy(y_sb[:, 1024:2048], pys[1][:])
                V.tensor_tensor(out=dsc[:], in0=gram[:], in1=idm[:],
                                op=OP.mult)
                V.tensor_reduce(st2[:, 0:1], dsc[:],
                                axis=mybir.AxisListType.X, op=OP.add)
                V.tensor_copy(y_sb[:, 2048:3072], pys[2][:])
                V.tensor_copy(y_sb[:, 3072:4096], pys[3][:])

                if prev is not None:
                    finish(prev, last=False)
                prev = (i, y_sb, st2, pys)

            finish(prev, last=True)
    nc.finalize()
    return nc


_NC = None


def _get_nc():
    global _NC
    if _NC is None:
        _NC = build_nc()
    return _NC


def _prep_weights(weights: np.ndarray) -> np.ndarray:
    # w_sb[p, kc*256 + j] = W[r, j, k], k = kc*128 + p, kc = 2r + c
    wt = (weights.astype(np.float32).transpose(0, 2, 1)      # [r, k, j]
          .reshape(R, 2, P, BS).transpose(2, 0, 1, 3)        # [p, r, c, j]
          .reshape(P, WCOLS))
    return np.ascontiguousarray(wt.astype(bfnp))


def _in_maps(x, weights):
    xf = np.ascontiguousarray(x, dtype=np.float32).reshape(ROWS_TOTAL, D)
    xb = xf.astype(bfnp)
    wid = _prep_weights(np.asarray(weights))
    idm = np.eye(P, dtype=np.float32)
    maps = []
    for c in range(N_CORES):
        xc = xb[c * ROWS_CORE:(c + 1) * ROWS_CORE]           # [1024, 4096]
        # xt[t, p, kc, row] = xc[t*128 + row, kc*128 + p]
        xt = np.ascontiguousarray(
            xc.reshape(NT, P, NCH, P).transpose(0, 3, 2, 1))
        maps.append({"xt": xt, "w": wid, "idm": idm})
    return maps


def kernel(x: np.ndarray, weights: np.ndarray) -> np.ndarray:
    nc = _get_nc()
    res = run_bass_kernel_spmd(nc, _in_maps(x, weights), list(range(N_CORES)))
    out = np.concatenate(
        [np.asarray(res.results[i]["out"]) for i in range(N_CORES)], axis=0)
    return out.reshape(x.shape).astype(np.float32)


def run_traced(x, weights, trace_dir):
    """test.py only: run with NTFF tracing, artifacts into trace_dir."""
    return run_bass_kernel_spmd(
        _get_nc(), _in_maps(x, weights), list(range(N_CORES)),
        trace=True, tmpdir=trace_dir)


if __name__ == "__main__":
    xs = np.random.randn(4, 2048, D).astype(np.float32)
    ws = (np.broadcast_to(np.eye(BS, dtype=np.float32), (16, BS, BS))
          + 0.02 * np.random.randn(16, BS, BS).astype(np.float32))
    o = kernel(xs, ws)
    print("kernel ran, out shape", o.shape, o.dtype)
